# revision 37
# baseline (speedup 1.0000x reference)
"""GraphTransformer 2-layer (TransformerConv x2) on 8 Trainium2 NeuronCores.

Sharding: destination-node partitioning with degree-sorted padded tiles.
  - Pad N=50000 -> N'=50176 (392 tiles of 128 nodes). Sort nodes by
    in-degree, bin-pack the tiles onto 8 cores (49 each, balancing slots).
  - Layer-1 K|V is computed per-core into GROUP TABLES: the core's 49 dst
    tiles are split into ~3 groups whose unique source sets stay under the
    int16 index limit; the host ships x columns for each group's unique
    sources and the device projects them to a bf16 K|V table per group.
    Each dst tile then pulls all its neighbors' K|V rows with ONE
    dma_gather (group-local int16 indices, one row per edge slot).
  - Features are stored c-major ([c,h]) so every big elementwise op keeps
    a packed 2-byte last dim and runs in the DVE 2x mode; reductions are
    tree-adds.  Padding slots point at group row 0 with a -1e30 logit
    bias, so exp() kills them (no max-subtraction pass).
  - Layer-2 K|V is computed from the local h chunk, packed two nodes per
    256B fp32 row (pair = tiles 2t/2t+1 at the same partition, so the
    packed table has NP/2 < 32768 rows and one int16 dma_gather per edge
    group suffices), AllGathered, gathered per edge, and the right half
    of each row is selected with a cheap per-slot parity blend.
All indices/degrees/tile shapes are baked in at build time from the actual
inputs. kernel() builds + runs the single-launch SPMD program and
unpermutes the output on the host.
"""

import numpy as np

N_CORES = 8
N = 50000
IN_DIM = 128
D1 = 256            # heads*hid layer1
H1, C1 = 8, 32
D2 = 10             # layer2 out channels (1 head)
KV2W = 64           # layer2 packed-pair row width (fp32) = 256B for dma_gather
P = 128
NEG = -1.0e30
U_CAP = 31744       # max rows per L1 group table (int16 headroom)
GCAP = 40           # max sum(D) per L2 gather group (desc-ring headroom)

_GROUPINFO = {}


def _plan(edge_index):
    src = np.asarray(edge_index[0], dtype=np.int64)
    dst = np.asarray(edge_index[1], dtype=np.int64)
    deg = np.bincount(dst, minlength=N)
    NP_ = ((N + N_CORES * P - 1) // (N_CORES * P)) * (N_CORES * P)  # 50176
    n_tiles = NP_ // P                                              # 392
    per_core = n_tiles // N_CORES                                   # 49

    degp = np.concatenate([deg, np.zeros(NP_ - N, np.int64)])
    order0 = np.argsort(degp, kind="stable")        # old(padded) ids, deg asc
    tile_of = order0.reshape(n_tiles, P)            # prelim tile -> old ids
    tile_D = degp[tile_of].max(axis=1)

    # bin-pack tiles onto cores: largest-first greedy with capacity
    t_order = np.argsort(-tile_D, kind="stable")
    loads = np.zeros(N_CORES, np.int64)
    counts = np.zeros(N_CORES, np.int64)
    assign = [[] for _ in range(N_CORES)]
    for t in t_order:
        open_cores = [c for c in range(N_CORES) if counts[c] < per_core]
        c = min(open_cores, key=lambda cc: (loads[cc], cc))
        assign[c].append(int(t))
        loads[c] += int(tile_D[t])
        counts[c] += 1
    for c in range(N_CORES):
        assign[c].sort(key=lambda t: int(tile_D[t]))

    final_tiles = [t for c in range(N_CORES) for t in assign[c]]
    perm = tile_of[final_tiles].reshape(-1)         # new id -> old(padded) id
    inv = np.empty(NP_, np.int64)
    inv[perm] = np.arange(NP_)

    Ds = degp[perm].reshape(n_tiles, P).max(axis=1).astype(np.int64)
    Ds_pos = [int(max(Ds[c * per_core + j] for c in range(N_CORES)))
              for j in range(per_core)]

    # per-(new)tile neighbor tables in NEW ids; pad idx=0, bias=NEG
    dst_new = inv[dst]
    src_new = inv[src]
    eo = np.argsort(dst_new, kind="stable")
    dst_s = dst_new[eo]
    src_s = src_new[eo]
    row_start = np.searchsorted(dst_s, np.arange(NP_))
    row_end = np.searchsorted(dst_s, np.arange(NP_) + 1)

    idx_tiles, bias_tiles = [], []
    for t in range(n_tiles):
        D = int(Ds[t])
        it = np.zeros((P, D), np.int64)
        bt = np.full((P, D), NEG, np.float32)
        for p in range(P):
            s, e = row_start[t * P + p], row_end[t * P + p]
            k = e - s
            it[p, :k] = src_s[s:e]
            bt[p, :k] = 0.0
        idx_tiles.append(it)
        bias_tiles.append(bt)

    # ---- L1 gather groups (position-aligned across cores) ----
    # grow each group while every core's unique-source count stays under
    # U_CAP; pad U_g to the max across cores (128-aligned).
    l1_groups = []      # (j0, j1, U_g, Goff)
    uniq_sets = None
    j0 = 0
    Goff = 0

    def _uniq_count(c, jlo, jhi):
        tiles = [idx_tiles[c * per_core + j][:, :Ds_pos[j]]
                 if Ds_pos[j] > Ds[c * per_core + j] else
                 idx_tiles[c * per_core + j]
                 for j in range(jlo, jhi)]
        # position-aligned D means idx arrays are Ds[t]-wide; the padded
        # positions reuse idx 0 which is force-included anyway.
        return np.unique(np.concatenate(
            [t.reshape(-1) for t in tiles] + [np.zeros(1, np.int64)]))

    j = 0
    while j < per_core:
        j1 = j
        while j1 < per_core:
            cand = j1 + 1
            maxu = max(len(_uniq_count(c, j, cand)) for c in range(N_CORES))
            if maxu > U_CAP and cand - j > 1:
                break
            j1 = cand
            if maxu > U_CAP:
                break
        U_g = max(len(_uniq_count(c, j, j1)) for c in range(N_CORES))
        U_g = ((U_g + P - 1) // P) * P
        assert U_g <= 32640
        l1_groups.append((j, j1, U_g, Goff))
        Goff += U_g
        j = j1
    UGtot = ((Goff + 1023) // 1024) * 1024

    # ---- L2 gather groups (desc-count capped) ----
    l2_groups = []
    j = 0
    while j < per_core:
        j1 = j
        tot = 0
        while j1 < per_core and (j1 == j or tot + Ds_pos[j1] <= GCAP):
            tot += Ds_pos[j1]
            j1 += 1
        l2_groups.append((j, j1))
        j = j1

    _GROUPINFO.clear()
    _GROUPINFO.update(dict(Ds_pos=Ds_pos, l1_groups=l1_groups,
                           l2_groups=l2_groups, UGtot=UGtot))

    return dict(NP=NP_, n_tiles=n_tiles, per_core=per_core, perm=perm,
                inv=inv, Ds=[int(d) for d in Ds], Ds_pos=Ds_pos,
                idx_tiles=idx_tiles, bias_tiles=bias_tiles,
                l1_groups=l1_groups, l2_groups=l2_groups, UGtot=UGtot)


def _wrap16(v):
    """Pack an index vector into the dma_gather SBUF layout:
    [128, len(v)/16] with entry (p, s) = v[s*16 + p%16]."""
    n = len(v)
    assert n % 16 == 0
    w = v.reshape(n // 16, 16)
    return np.ascontiguousarray(w[:, np.arange(P) % 16].T)


def _build_program(NP_, per_core, Ds_pos, biases_zero, sim1=False):
    import concourse.bass as bass  # noqa: F401
    import concourse.mybir as mybir
    from concourse import bacc
    from concourse.tile import TileContext
    from concourse.masks import make_identity

    gi = _GROUPINFO
    assert gi and gi["Ds_pos"] == list(Ds_pos), "call _plan() first"
    l1_groups = gi["l1_groups"]
    l2_groups = gi["l2_groups"]
    UGtot = gi["UGtot"]

    f32 = mybir.dt.float32
    bf16 = mybir.dt.bfloat16
    i16 = mybir.dt.int16
    NOWN = per_core * P
    Dmax = max(Ds_pos)
    totD = sum(Ds_pos)
    cum = [0]
    for d in Ds_pos:
        cum.append(cum[-1] + d)
    l1_group_of = {}
    for (j0, j1, U_g, Goff) in l1_groups:
        for j in range(j0, j1):
            l1_group_of[j] = (U_g, Goff)

    nc = bacc.Bacc("TRN2", target_bir_lowering=False, debug=False,
                   num_devices=1 if sim1 else N_CORES)

    xqT = nc.dram_tensor("xqT", [IN_DIM, NOWN], bf16, kind="ExternalInput")
    xgT = nc.dram_tensor("xgT", [IN_DIM, UGtot], bf16, kind="ExternalInput")
    w_kv1 = nc.dram_tensor("w_kv1", [IN_DIM, 2 * D1], bf16, kind="ExternalInput")
    w_qs1 = nc.dram_tensor("w_qs1", [IN_DIM, 2 * D1], bf16, kind="ExternalInput")
    # layer-2 weights packed as [kv2_c0 | qs2_c0] ; [kv2_c1 | qs2_c1]
    w2a = nc.dram_tensor("w2a", [P, 2 * (2 * D2)], bf16, kind="ExternalInput")
    w2b = nc.dram_tensor("w2b", [P, 2 * (2 * D2)], bf16, kind="ExternalInput")
    b_kv1 = nc.dram_tensor("b_kv1", [1, 2 * D1], bf16, kind="ExternalInput")
    b_qs1 = nc.dram_tensor("b_qs1", [1, 2 * D1], bf16, kind="ExternalInput")
    b2row = nc.dram_tensor("b2row", [1, 2 * (2 * D2)], bf16,
                           kind="ExternalInput")
    idx16_pm = nc.dram_tensor("idx16_pm", [P, totD * 8], i16,
                              kind="ExternalInput")
    idx2_pm = nc.dram_tensor("idx2_pm", [P, totD * 8], i16,
                             kind="ExternalInput")
    par_pm = nc.dram_tensor("par_pm", [P, totD], bf16, kind="ExternalInput")
    bias_pm = nc.dram_tensor("bias_pm", [P, totD], bf16, kind="ExternalInput")
    out_d = nc.dram_tensor("out", [NOWN, D2], f32, kind="ExternalOutput")

    kv1_g = nc.dram_tensor("kv1_g", [UGtot, 2 * D1], bf16)
    qs1_t = nc.dram_tensor("qs1_t", [NOWN, 2 * D1], bf16)
    # layer-2 table packs node pairs (tile 2t, tile 2t+1 @ same partition)
    # into one 256B row so one int16 dma_gather covers all 50176 nodes
    TPAIR = (per_core + 1) // 2
    NOWNP = TPAIR * P
    kv2_own = nc.dram_tensor("kv2_own", [NOWNP, KV2W], f32)
    kv2_full = nc.dram_tensor("kv2_full", [N_CORES * NOWNP, KV2W], f32,
                              addr_space="Shared")

    X = mybir.AxisListType.X
    MUL = mybir.AluOpType.mult
    ADD = mybir.AluOpType.add
    EXP = mybir.ActivationFunctionType.Exp

    with nc.allow_low_precision(reason="bf16 attention within 2e-2 tolerance"), \
         TileContext(nc) as tc:
        with tc.tile_pool(name="wpool", bufs=1) as wpool:
            w_kv1_s = wpool.tile([IN_DIM, 2 * D1], bf16)
            nc.sync.dma_start(out=w_kv1_s[:], in_=w_kv1[:, :])
            w_qs1_s = wpool.tile([IN_DIM, 2 * D1], bf16)
            nc.sync.dma_start(out=w_qs1_s[:], in_=w_qs1[:, :])
            w2a_s = wpool.tile([P, 2 * (2 * D2)], bf16)
            nc.sync.dma_start(out=w2a_s[:], in_=w2a[:, :])
            w2b_s = wpool.tile([P, 2 * (2 * D2)], bf16)
            nc.sync.dma_start(out=w2b_s[:], in_=w2b[:, :])
            idx16_sb = wpool.tile([P, totD * 8], i16)
            nc.sync.dma_start(out=idx16_sb[:], in_=idx16_pm[:, :])
            biasm_sb = wpool.tile([P, totD], bf16)
            nc.sync.dma_start(out=biasm_sb[:], in_=bias_pm[:, :])
            ones1 = wpool.tile([1, P], bf16)
            nc.vector.memset(ones1[:], 1.0)
            b2_s = wpool.tile([1, 2 * (2 * D2)], bf16)
            nc.sync.dma_start(out=b2_s[:], in_=b2row[:, :])
            if not biases_zero:
                b_kv1_s = wpool.tile([1, 2 * D1], bf16)
                nc.sync.dma_start(out=b_kv1_s[:], in_=b_kv1[:, :])
                b_qs1_s = wpool.tile([1, 2 * D1], bf16)
                nc.sync.dma_start(out=b_qs1_s[:], in_=b_qs1[:, :])
            ident = wpool.tile([P, P], bf16)
            make_identity(nc, ident[:])
            # SBUF-resident per-core state
            kv2_sb = wpool.tile([P, per_core * 2 * D2], f32)    # K|V layer2
            qs2_sb = wpool.tile([P, per_core * 2 * D2], f32)    # Q|S layer2
            out_sb = wpool.tile([P, per_core * D2], f32)        # final out

            # ================= P1a: own-node Q|S projections =================
            with tc.tile_pool(name="p1q", bufs=1) as p1q, \
                 tc.tile_pool(name="p1qo", bufs=4) as p1qo, \
                 tc.tile_pool(name="p1qps", bufs=8, space="PSUM") as p1qps:
                xq_s = p1q.tile([P, NOWN], bf16)
                nc.sync.dma_start(out=xq_s[:], in_=xqT[:, :])
                for t in range(per_core):
                    ps2 = p1qps.tile([P, 2 * D1], f32, tag="ps")
                    nc.tensor.matmul(out=ps2[:], lhsT=xq_s[:, t * P:(t + 1) * P],
                                     rhs=w_qs1_s[:], start=True,
                                     stop=biases_zero)
                    if not biases_zero:
                        nc.tensor.matmul(out=ps2[:], lhsT=ones1[:],
                                         rhs=b_qs1_s[:], start=False, stop=True)
                    qst = p1qo.tile([P, 2 * D1], bf16, tag="qs")
                    if t % 2 == 0:
                        nc.vector.tensor_copy(out=qst[:], in_=ps2[:])
                    else:
                        nc.scalar.copy(out=qst[:], in_=ps2[:])
                    nc.sync.dma_start(out=qs1_t[t * P:(t + 1) * P, :],
                                      in_=qst[:])

            # ================= P1b: group-table K|V projections ==============
            TB = 8                     # tiles per x block
            XB = TB * P                # 1024 columns per load
            with tc.tile_pool(name="p1x", bufs=6) as p1x, \
                 tc.tile_pool(name="p1ps", bufs=8, space="PSUM") as p1ps, \
                 tc.tile_pool(name="p1o", bufs=6) as p1o:
                for blk in range(UGtot // XB):
                    xT_s = p1x.tile([P, XB], bf16, tag="xT")
                    nc.sync.dma_start(out=xT_s[:],
                                      in_=xgT[:, blk * XB:(blk + 1) * XB])
                    kvstage = p1o.tile([P, TB * 2 * D1], bf16, tag="kv")
                    for jj in range(TB):
                        t = blk * TB + jj
                        lhsT = xT_s[:, jj * P:(jj + 1) * P]
                        ps = p1ps.tile([P, 2 * D1], f32, tag="ps")
                        nc.tensor.matmul(out=ps[:], lhsT=lhsT, rhs=w_kv1_s[:],
                                         start=True, stop=biases_zero)
                        if not biases_zero:
                            nc.tensor.matmul(out=ps[:], lhsT=ones1[:],
                                             rhs=b_kv1_s[:], start=False,
                                             stop=True)
                        dst_ap = kvstage[:, jj * 2 * D1:(jj + 1) * 2 * D1]
                        # GPSIMD cannot read PSUM: copies go DVE/ACT only
                        if t % 2 == 0:
                            nc.scalar.copy(out=dst_ap, in_=ps[:])
                        else:
                            nc.vector.tensor_copy(out=dst_ap, in_=ps[:])
                    half = TB // 2
                    nc.sync.dma_start(
                        out=kv1_g[blk * XB:blk * XB + half * P, :]
                            .rearrange("(t p) f -> p t f", p=P),
                        in_=kvstage[:, 0:half * 2 * D1]
                            .rearrange("p (t f) -> p t f", t=half))
                    nc.sync.dma_start(
                        out=kv1_g[blk * XB + half * P:(blk + 1) * XB, :]
                            .rearrange("(t p) f -> p t f", p=P),
                        in_=kvstage[:, half * 2 * D1:]
                            .rearrange("p (t f) -> p t f", t=half))

            # ========== P2: layer-1 attention + layer-2 projections ==========
            def _pd(D):
                return (D * 3) // 10 if D >= 4 else 0
            with tc.tile_pool(name="kvb", bufs=4) as kvb, \
                 tc.tile_pool(name="qsb", bufs=4) as qsb, \
                 tc.tile_pool(name="small", bufs=3) as small, \
                 tc.tile_pool(name="hps", bufs=2, space="PSUM") as hps, \
                 tc.tile_pool(name="hps2", bufs=2, space="PSUM") as hps2, \
                 tc.tile_pool(name="houtp", bufs=2) as houtp:
                gathered = {}

                def p2_gather(j):
                    D = Ds_pos[j]
                    c0 = cum[j]
                    U_g, Goff = l1_group_of[j]
                    kv_s = kvb.tile([P, Dmax * 2 * D1], bf16, tag="kv")
                    nc.gpsimd.dma_gather(
                        kv_s[:, 0:D * 2 * D1].rearrange(
                            "p (n f) -> p n f", n=D),
                        kv1_g[Goff:Goff + U_g, :],
                        idx16_sb[:, c0 * 8:(c0 + D) * 8],
                        D * P,
                        D * P,
                        2 * D1,
                        single_packet=False,
                    )
                    qs_s = qsb.tile([P, 2 * D1], bf16, tag="qs")
                    nc.sync.dma_start(out=qs_s[:],
                                      in_=qs1_t[j * P:(j + 1) * P, :])
                    gathered[j] = (kv_s, qs_s)

                def p2_head(j):
                    D = Ds_pos[j]
                    c0 = cum[j]
                    # d-range split: DVE handles [0:dv); Pool handles [dv:D)
                    # (disjoint ranges of one tile run in parallel)
                    pd = _pd(D)
                    dv = D - pd
                    kv_s, qs_s = gathered.pop(j)
                    kv5 = kv_s[:, 0:D * 2 * D1].rearrange(
                        "p (d x c h) -> p d x c h", d=D, x=2, c=C1)
                    K4 = kv5[:, :, 0, :, :]              # [P, D, 32, 8] bf16
                    q3 = qs_s[:, 0:D1].rearrange("p (c h) -> p c h", c=C1)
                    # logits: q . k in-place on the K region (bf16 2x mode)
                    P4 = K4[:, 0:dv]
                    P4p = K4[:, dv:D] if pd else None
                    nc.vector.tensor_tensor(
                        out=P4, in0=P4,
                        in1=q3.unsqueeze(1).to_broadcast([P, dv, C1, H1]),
                        op=MUL)
                    if pd:
                        nc.gpsimd.tensor_tensor(
                            out=P4p, in0=P4p,
                            in1=q3.unsqueeze(1).to_broadcast([P, pd, C1, H1]),
                            op=MUL)
                    cur = C1
                    while cur > 1:
                        nh = cur // 2
                        nc.vector.tensor_tensor(
                            out=P4[:, :, 0:nh, :], in0=P4[:, :, 0:nh, :],
                            in1=P4[:, :, cur - nh:cur, :], op=ADD)
                        if pd:
                            nc.gpsimd.tensor_tensor(
                                out=P4p[:, :, 0:nh, :],
                                in0=P4p[:, :, 0:nh, :],
                                in1=P4p[:, :, cur - nh:cur, :], op=ADD)
                        cur = nh
                    return dict(j=j, D=D, c0=c0, dv=dv, pd=pd, kv_s=kv_s,
                                qs_s=qs_s, P4=P4, P4p=P4p)

                def p2_head_b(st):
                    D, c0, dv, pd = st["D"], st["c0"], st["dv"], st["pd"]
                    P4, P4p = st["P4"], st["P4p"]
                    # finalize logits in fp32 with padding bias
                    lg = small.tile([P, Dmax * H1], f32, tag="lg")
                    nc.vector.tensor_tensor(
                        out=lg[:, 0:dv * H1].rearrange("p (d h) -> p d h", d=dv),
                        in0=P4[:, :, 0, :],
                        in1=biasm_sb[:, c0:c0 + dv].unsqueeze(2)
                            .to_broadcast([P, dv, H1]),
                        op=ADD)
                    if pd:
                        nc.vector.tensor_tensor(
                            out=lg[:, dv * H1:D * H1].rearrange(
                                "p (d h) -> p d h", d=pd),
                            in0=P4p[:, :, 0, :],
                            in1=biasm_sb[:, c0 + dv:c0 + D].unsqueeze(2)
                                .to_broadcast([P, pd, H1]),
                            op=ADD)
                    # softmax (no max-subtraction: logits are O(10) bounded;
                    # pad slots have -1e30 -> exp == 0)
                    e_bf = small.tile([P, Dmax * H1], bf16, tag="ebf")
                    nc.scalar.activation(out=e_bf[:, 0:D * H1],
                                         in_=lg[:, 0:D * H1], func=EXP)
                    st["e_bf"] = e_bf

                def p2_tail(st):
                    j, D = st["j"], st["D"]
                    kv_s, e_bf = st["kv_s"], st["e_bf"]
                    qs_s = st["qs_s"]
                    kv5 = kv_s[:, 0:D * 2 * D1].rearrange(
                        "p (d x c h) -> p d x c h", d=D, x=2, c=C1)
                    V4 = kv5[:, :, 1, :, :]
                    V3 = kv_s[:, 0:D * 2 * D1].rearrange(
                        "p (d x f) -> p d x f", d=D, x=2)[:, :, 1, :]
                    sm = small.tile([P, H1], f32, tag="sm")
                    nc.vector.reduce_sum(
                        out=sm[:],
                        in_=e_bf[:, 0:D * H1].rearrange("p (d h) -> p h d", d=D),
                        axis=X)
                    nc.vector.tensor_scalar_add(out=sm[:], in0=sm[:],
                                                scalar1=1e-16)
                    rc = small.tile([P, H1], f32, tag="rc")
                    nc.vector.reciprocal(out=rc[:], in_=sm[:])
                    rcb = small.tile([P, H1], bf16, tag="rcb")
                    nc.vector.tensor_copy(out=rcb[:], in_=rc[:])
                    # weighted V: V *= e  (bf16 2x), then tree-sum over d
                    E3 = e_bf[:, 0:D * H1].rearrange("p (d h) -> p d h", d=D)
                    nc.vector.tensor_tensor(
                        out=V4, in0=V4,
                        in1=E3.unsqueeze(2).to_broadcast([P, D, C1, H1]),
                        op=MUL)
                    cur = D
                    while cur > 1:
                        nh = cur // 2
                        nc.vector.tensor_tensor(
                            out=V3[:, 0:nh, :], in0=V3[:, 0:nh, :],
                            in1=V3[:, cur - nh:cur, :], op=ADD)
                        cur = cur - nh
                    # z = attention + skip, all bf16 from here
                    att = houtp.tile([P, D1], bf16, tag="att")
                    A3 = att[:].rearrange("p (c h) -> p c h", c=C1)
                    nc.vector.tensor_tensor(
                        out=A3,
                        in0=V3[:, 0, :].rearrange("p (c h) -> p c h", c=C1),
                        in1=rcb[:].unsqueeze(1).to_broadcast([P, C1, H1]),
                        op=MUL)
                    nc.vector.tensor_tensor(
                        out=att[:], in0=att[:],
                        in1=qs_s[:, D1:2 * D1],
                        op=ADD)
                    # ELU (minus the -1, folded into the layer-2 bias row):
                    # h+1 = relu(z) + exp(min(z,0))
                    zmin = houtp.tile([P, D1], bf16, tag="zmin")
                    nc.vector.tensor_scalar_min(out=zmin[:], in0=att[:],
                                                scalar1=0.0)
                    ez = houtp.tile([P, D1], bf16, tag="ez")
                    nc.scalar.activation(out=ez[:], in_=zmin[:], func=EXP)
                    nc.vector.tensor_sub(out=att[:], in0=att[:], in1=zmin[:])
                    h_bf = houtp.tile([P, D1], bf16, tag="h")
                    nc.vector.tensor_add(out=h_bf[:], in0=att[:], in1=ez[:])

                    # ---- layer-2 projections for this tile ----
                    hT0 = hps.tile([P, P], bf16, tag="hT")
                    nc.tensor.transpose(out=hT0[:], in_=h_bf[:, 0:P],
                                        identity=ident[:])
                    hT0s = houtp.tile([P, P], bf16, tag="hT0s")
                    nc.scalar.copy(out=hT0s[:], in_=hT0[:])
                    hT1 = hps.tile([P, P], bf16, tag="hT")
                    nc.tensor.transpose(out=hT1[:], in_=h_bf[:, P:2 * P],
                                        identity=ident[:])
                    hT1s = houtp.tile([P, P], bf16, tag="hT1s")
                    nc.scalar.copy(out=hT1s[:], in_=hT1[:])
                    ps2 = hps2.tile([P, 2 * (2 * D2)], f32, tag="ps2")
                    nc.tensor.matmul(out=ps2[:], lhsT=hT0s[:], rhs=w2a_s[:],
                                     start=True, stop=False)
                    nc.tensor.matmul(out=ps2[:], lhsT=hT1s[:], rhs=w2b_s[:],
                                     start=False, stop=False)
                    # bias row: absorbs layer-2 biases and the ELU "-1"
                    nc.tensor.matmul(out=ps2[:], lhsT=ones1[:], rhs=b2_s[:],
                                     start=False, stop=True)
                    nc.scalar.copy(out=kv2_sb[:, j * 2 * D2:(j + 1) * 2 * D2],
                                   in_=ps2[:, 0:2 * D2])
                    nc.scalar.copy(out=qs2_sb[:, j * 2 * D2:(j + 1) * 2 * D2],
                                   in_=ps2[:, 2 * D2:4 * D2])

                p2_gather(0)
                p2_gather(1)
                prev = None
                for j in range(per_core):
                    if j + 2 < per_core:
                        p2_gather(j + 2)
                    st = p2_head(j)
                    if prev is not None:
                        p2_tail(prev)
                    p2_head_b(st)
                    prev = st
                p2_tail(prev)
                # packed write: row (t*128+p) cols 0:40 <- tiles 2t,2t+1 at
                # partition p (the trailing odd tile's partner is garbage and
                # is never referenced)
                nc.sync.dma_start(
                    out=kv2_own[:, 0:4 * D2].rearrange(
                        "(t p) f -> p t f", p=P)[:, 0:per_core // 2, :],
                    in_=kv2_sb[:, 0:(per_core // 2) * 4 * D2].rearrange(
                        "p (t f) -> p t f", t=per_core // 2))
                if per_core % 2:
                    nc.sync.dma_start(
                        out=kv2_own[:, 0:2 * D2].rearrange(
                            "(t p) f -> p t f", p=P)[:, TPAIR - 1:TPAIR, :],
                        in_=kv2_sb[:, (per_core - 1) * 2 * D2:].rearrange(
                            "p (t f) -> p t f", t=1))
                    # zero the missing pair partner's half so the parity
                    # blend never multiplies NaN garbage by zero
                    zpad = houtp.tile([P, 2 * D2], f32, tag="zpad")
                    nc.vector.memset(zpad[:], 0.0)
                    nc.sync.dma_start(
                        out=kv2_own[:, 2 * D2:4 * D2].rearrange(
                            "(t p) f -> p t f", p=P)[:, TPAIR - 1:TPAIR, :],
                        in_=zpad[:].rearrange("p (t f) -> p t f", t=1))

            # ================= P4: AllGather kv2 =================
            if sim1:
                for c in range(N_CORES):
                    nc.sync.dma_start(
                        out=kv2_full[c * NOWNP:(c + 1) * NOWNP, :],
                        in_=kv2_own[:, :])
            else:
                nc.gpsimd.collective_compute(
                    "AllGather", mybir.AluOpType.bypass,
                    replica_groups=[list(range(N_CORES))],
                    ins=[kv2_own.ap().opt()],
                    outs=[kv2_full.ap().opt()],
                )

            # ================= P5: layer-2 attention =================
            # packed rows: [nodeE K(10) V(10) | nodeO K(10) V(10) | pad(24)]
            gd_max = max(cum[j1] - cum[j0] for j0, j1 in l2_groups)
            n_groups = len(l2_groups)
            group_of = {}
            for g, (j0, j1) in enumerate(l2_groups):
                for jj in range(j0, j1):
                    group_of[jj] = g
            XY = mybir.AxisListType.XY
            with tc.tile_pool(name="p5idx", bufs=1) as p5idx, \
                 tc.tile_pool(name="kvb2", bufs=3) as kvb2, \
                 tc.tile_pool(name="small2", bufs=6) as small2:
                idx2_sb = p5idx.tile([P, totD * 8], i16)
                nc.sync.dma_start(out=idx2_sb[:], in_=idx2_pm[:, :])
                par_sb = p5idx.tile([P, totD], bf16)
                nc.sync.dma_start(out=par_sb[:], in_=par_pm[:, :])
                group_tiles = {}

                def p5_gather(g):
                    j0, j1 = l2_groups[g]
                    gc0 = cum[j0]
                    gd = cum[j1] - gc0
                    kv2g = kvb2.tile([P, gd_max * KV2W], f32, tag="kv2")
                    nc.gpsimd.dma_gather(
                        kv2g[:, 0:gd * KV2W].rearrange(
                            "p (n f) -> p n f", n=gd),
                        kv2_full[:, :],
                        idx2_sb[:, gc0 * 8:(gc0 + gd) * 8],
                        gd * P, gd * P, KV2W, single_packet=False)
                    group_tiles[g] = kv2g

                def p5_head(j):
                    D = Ds_pos[j]
                    c0 = cum[j]
                    g = group_of[j]
                    off = c0 - cum[l2_groups[g][0]]
                    kv2g = group_tiles[g]
                    G4 = kv2g[:, off * KV2W:(off + D) * KV2W].rearrange(
                        "p (d f) -> p d f", d=D)
                    # [P, D, 2, 20]: x = node half, last 20 = [K(10) V(10)]
                    G40 = G4[:, :, 0:4 * D2].rearrange(
                        "p d (x c) -> p d x c", x=2)
                    prod2 = small2.tile([P, Dmax * 2 * D2], f32, tag="p2")
                    P4_ = prod2[:, 0:D * 2 * D2].rearrange(
                        "p (d x c) -> p d x c", d=D, x=2)
                    q2b = qs2_sb[:, j * 2 * D2:j * 2 * D2 + D2] \
                        .unsqueeze(1).unsqueeze(1).to_broadcast([P, D, 2, D2])
                    nc.vector.tensor_tensor(
                        out=P4_, in0=G40[:, :, :, 0:D2],
                        in1=q2b, op=MUL)
                    lg2x = small2.tile([P, Dmax * 2], f32, tag="lg2x")
                    nc.vector.reduce_sum(out=lg2x[:, 0:D * 2], in_=P4_, axis=X)
                    L2v = lg2x[:, 0:D * 2].rearrange("p (d x) -> p d x", x=2)
                    lg2 = small2.tile([P, Dmax], f32, tag="lg2")
                    # parity blend: lg = lgE + (lgO - lgE) * par
                    nc.vector.tensor_sub(out=L2v[:, :, 1], in0=L2v[:, :, 1],
                                         in1=L2v[:, :, 0])
                    nc.vector.tensor_tensor(
                        out=L2v[:, :, 1], in0=L2v[:, :, 1],
                        in1=par_sb[:, c0:c0 + D], op=MUL)
                    nc.vector.tensor_add(out=lg2[:, 0:D], in0=L2v[:, :, 0],
                                         in1=L2v[:, :, 1])
                    nc.vector.tensor_tensor(
                        out=lg2[:, 0:D], in0=lg2[:, 0:D],
                        in1=biasm_sb[:, c0:c0 + D], op=ADD)
                    e2 = small2.tile([P, Dmax], f32, tag="e2")
                    sm2 = small2.tile([P, 1], f32, tag="sm2")
                    nc.scalar.activation(out=e2[:, 0:D], in_=lg2[:, 0:D],
                                         func=EXP, accum_out=sm2[:])
                    return dict(j=j, D=D, c0=c0, G4=G4, e2=e2, sm2=sm2)

                def p5_tail(st):
                    j, D, c0, G4, e2 = (st["j"], st["D"], st["c0"], st["G4"],
                                        st["e2"])
                    sm2 = st["sm2"]
                    nc.vector.tensor_scalar_add(out=sm2[:], in0=sm2[:],
                                                scalar1=1e-16)
                    rc2 = small2.tile([P, 1], f32, tag="rc2")
                    nc.vector.reciprocal(out=rc2[:], in_=sm2[:])
                    # split weights by parity: eO = e2*par, eE = e2 - eO
                    eEO = small2.tile([P, Dmax * 2], f32, tag="eEO")
                    E2v = eEO[:, 0:D * 2].rearrange("p (d x) -> p d x", x=2)
                    nc.vector.tensor_tensor(
                        out=E2v[:, :, 1], in0=e2[:, 0:D],
                        in1=par_sb[:, c0:c0 + D], op=MUL)
                    nc.vector.tensor_sub(out=E2v[:, :, 0], in0=e2[:, 0:D],
                                         in1=E2v[:, :, 1])
                    # weighted V halves (cols 10:20 and 30:40), in place
                    Vh = G4[:, :, 0:4 * D2].rearrange(
                        "p d (x c) -> p d x c", x=2)[:, :, :, D2:2 * D2]
                    nc.vector.tensor_tensor(
                        out=Vh, in0=Vh,
                        in1=E2v.unsqueeze(3).to_broadcast([P, D, 2, D2]),
                        op=MUL)
                    att2 = small2.tile([P, D2], f32, tag="att2")
                    nc.vector.reduce_sum(
                        out=att2[:],
                        in_=Vh.transpose([0, 3, 1, 2]),
                        axis=XY)
                    nc.vector.tensor_scalar_mul(out=att2[:], in0=att2[:],
                                                scalar1=rc2[:])
                    nc.vector.tensor_tensor(
                        out=out_sb[:, j * D2:(j + 1) * D2], in0=att2[:],
                        in1=qs2_sb[:, j * 2 * D2 + D2:(j + 1) * 2 * D2],
                        op=ADD)

                p5_gather(0)
                prev = None
                for j in range(per_core):
                    g = group_of[j]
                    if j == l2_groups[g][0] and g + 1 < n_groups:
                        p5_gather(g + 1)
                    st = p5_head(j)
                    if prev is not None:
                        p5_tail(prev)
                    prev = st
                p5_tail(prev)
                nc.sync.dma_start(
                    out=out_d[:, :].rearrange("(t p) f -> p t f", p=P),
                    in_=out_sb[:].rearrange("p (t f) -> p t f", t=per_core))

    nc.compile()
    return nc


_CACHE = {}


def _get_program(NP_, per_core, Ds_pos, biases_zero):
    key = (NP_, per_core, tuple(Ds_pos), biases_zero)
    if key not in _CACHE:
        _CACHE[key] = _build_program(NP_, per_core, Ds_pos, biases_zero)
    return _CACHE[key]


def _cmajor_cols():
    # new column (c*8+h) <- old column (h*32+c)
    return (np.arange(D1).reshape(H1, C1)).T.reshape(-1)


def kernel(**inputs):
    import ml_dtypes
    from concourse.bass_utils import run_bass_kernel_spmd

    bf = ml_dtypes.bfloat16
    x = np.asarray(inputs["x"], np.float32)
    edge_index = np.asarray(inputs["edge_index"])
    plan = _plan(edge_index)
    NP_ = plan["NP"]
    per_core = plan["per_core"]
    Ds = plan["Ds"]
    Ds_pos = plan["Ds_pos"]
    NOWN = per_core * P
    totD = sum(Ds_pos)
    cum = np.zeros(per_core + 1, np.int64)
    cum[1:] = np.cumsum(Ds_pos)
    l1_groups = plan["l1_groups"]
    l2_groups = plan["l2_groups"]
    UGtot = plan["UGtot"]

    cm = _cmajor_cols()
    s1 = 1.0 / np.sqrt(np.float32(C1))
    s2 = 1.0 / np.sqrt(np.float32(D2))
    w1k = np.asarray(inputs["w1k"], np.float32)[:, cm]
    w1v = np.asarray(inputs["w1v"], np.float32)[:, cm]
    w1q = np.asarray(inputs["w1q"], np.float32)[:, cm] * s1
    w1s = np.asarray(inputs["w1s"], np.float32)[:, cm]
    w_kv1 = np.ascontiguousarray(
        np.concatenate([w1k, w1v], axis=1)).astype(bf)
    w_qs1 = np.ascontiguousarray(
        np.concatenate([w1q, w1s], axis=1)).astype(bf)
    # layer-2 weights: rows permuted to c-major (h is c-major), packed as
    # [kv2 | qs2] per 128-row chunk
    w2k = np.asarray(inputs["w2k"], np.float32)[cm, :]
    w2v = np.asarray(inputs["w2v"], np.float32)[cm, :]
    w2q = np.asarray(inputs["w2q"], np.float32)[cm, :] * s2
    w2s = np.asarray(inputs["w2s"], np.float32)[cm, :]
    wkv2 = np.concatenate([w2k, w2v], axis=1)      # [256, 20]
    wqs2 = np.concatenate([w2q, w2s], axis=1)      # [256, 20]
    w2a = np.ascontiguousarray(
        np.concatenate([wkv2[0:P], wqs2[0:P]], axis=1)).astype(bf)
    w2b = np.ascontiguousarray(
        np.concatenate([wkv2[P:2 * P], wqs2[P:2 * P]], axis=1)).astype(bf)
    b_kv1 = np.concatenate([np.asarray(inputs["b1k"], np.float32)[cm],
                            np.asarray(inputs["b1v"], np.float32)[cm]])[None]
    b_qs1 = np.concatenate([np.asarray(inputs["b1q"], np.float32)[cm] * s1,
                            np.asarray(inputs["b1s"], np.float32)[cm]])[None]
    b_kv2 = np.concatenate([np.asarray(inputs["b2k"], np.float32),
                            np.asarray(inputs["b2v"], np.float32)])[None]
    b_qs2 = np.concatenate([np.asarray(inputs["b2q"], np.float32) * s2,
                            np.asarray(inputs["b2s"], np.float32)])[None]
    # layer-2 bias row: external biases minus column sums of W2 (the device
    # computes (h+1) @ W2; subtracting sum_rows(W2) restores h @ W2 + b2)
    b2row = np.concatenate([b_kv2[0] - wkv2.sum(axis=0),
                            b_qs2[0] - wqs2.sum(axis=0)])[None]
    biases_zero = all(not np.any(b) for b in (b_kv1, b_qs1))

    nc = _get_program(NP_, per_core, Ds_pos, biases_zero)

    xpad = np.concatenate([x, np.zeros((NP_ - N, IN_DIM), np.float32)])
    x_new = xpad[plan["perm"]]                     # [NP, 128] new order
    x_new_bf = x_new.astype(bf)

    in_maps = []
    for c in range(N_CORES):
        own0 = c * NOWN
        xq = np.ascontiguousarray(x_new_bf[own0:own0 + NOWN].T)

        xg = np.zeros((IN_DIM, UGtot), bf)
        uniq_of_group = []
        for (j0, j1, U_g, Goff) in l1_groups:
            pieces = [plan["idx_tiles"][c * per_core + j].reshape(-1)
                      for j in range(j0, j1)]
            uniq = np.unique(np.concatenate(pieces + [np.zeros(1, np.int64)]))
            assert len(uniq) <= U_g
            uniq_of_group.append(uniq)
            xg[:, Goff:Goff + len(uniq)] = x_new_bf[uniq].T

        idx16 = np.zeros((P, totD * 8), np.int16)
        idx2 = np.zeros((P, totD * 8), np.int16)
        parm = np.zeros((P, totD), np.float32)
        biasm = np.full((P, totD), NEG, np.float32)
        TPAIR = (per_core + 1) // 2
        NOWNP = TPAIR * P
        for gi_, (j0, j1, U_g, Goff) in enumerate(l1_groups):
            uniq = uniq_of_group[gi_]
            for j in range(j0, j1):
                t_new = c * per_core + j
                D = Ds[t_new]
                Dp = Ds_pos[j]
                it = np.zeros((P, Dp), np.int64)
                it[:, :D] = plan["idx_tiles"][t_new]
                local = np.searchsorted(uniq, it)
                assert (uniq[local] == it).all()
                v = local.T.reshape(-1)            # i = d*128+p
                idx16[:, cum[j] * 8:(cum[j] + Dp) * 8] = _wrap16(
                    v.astype(np.int16))
                # layer-2 packed rows: global a -> core c', local tile jl,
                # packed row c'*NOWNP + (jl//2)*128 + p, half = jl%2
                ac = it // NOWN
                ar = it - ac * NOWN
                jl = ar // P
                pp = ar % P
                rp = ac * NOWNP + (jl // 2) * P + pp
                assert rp.max() < N_CORES * NOWNP <= 32768
                idx2[:, cum[j] * 8:(cum[j] + Dp) * 8] = _wrap16(
                    rp.T.reshape(-1).astype(np.int16))
                parm[:, cum[j]:cum[j] + Dp] = (jl % 2).astype(np.float32)
                biasm[:, cum[j]:cum[j] + D] = plan["bias_tiles"][t_new]
        in_maps.append(dict(
            xqT=xq, xgT=xg,
            w_kv1=w_kv1, w_qs1=w_qs1, w2a=w2a, w2b=w2b,
            b_kv1=b_kv1.astype(bf), b_qs1=b_qs1.astype(bf),
            b2row=b2row.astype(bf),
            idx16_pm=idx16, idx2_pm=idx2, par_pm=parm.astype(bf),
            bias_pm=biasm.astype(bf),
        ))

    res = run_bass_kernel_spmd(nc, in_maps, core_ids=list(range(N_CORES)))
    kernel.last_results = res

    out_new = np.concatenate([np.asarray(res.results[c]["out"])
                              for c in range(N_CORES)])
    mask = plan["perm"] < N
    out = np.empty((N, D2), np.float32)
    out[plan["perm"][mask]] = out_new[mask]
    return out
